# revision 1
# baseline (speedup 1.0000x reference)
"""Cross-attention Trainium2 kernel (Bass/Tile), data-parallel over batch on 8 cores.

Reference computation per batch element b (no 1/sqrt(d) scaling):
    Q = S2[b] @ Wq            [N2, E]
    K = S1[b] @ Wk            [N1, E]
    V = S1[b] @ Wv            [N1, E]
    A = softmax(Q @ K^T, -1)  [N2, N1]
    out[b] = (A @ V) @ Wo + bo  [N2, D]

Device layout is fully transposed (feature dims on SBUF partitions):
    host supplies S1T = S1[b].T, S2T = S2[b].T  [D, N]
    phase A: KT[e, m], V[m, e] -> DRAM scratch (float32r)
    phase B per 512-query chunk:
      QT chunk -> scoresT tiles [m-part, n-free] -> exp (no max subtraction:
      |score| <= ~70 and exp stays in fp32 range) -> ones-matmul row sums
      accumulated in PSUM -> reciprocal -> gpsimd partition_broadcast ->
      UT = V.T @ E accumulated in PSUM (two 4-bank passes), normalized during
      PSUM eviction -> outT = Wo.T @ maskedT + bo -> DRAM [D, N2]; host
      transposes back.

All matmul operands are float32r (TF32-like 12-bit-mantissa rounding inside
the PE, full throughput at moving dim >= 256, ~1.6e-4 matmul rel err).
"""
import sys

sys.path.insert(0, "/opt/trn_rl_repo")

import numpy as np
from contextlib import ExitStack

P = 128
N_CORES = 8
B = 8          # batch (one element per core)
NQ = 2048      # queries (N2)
NK = 2048      # keys (N1)
D = 512        # query/cross dim
EI = 1024      # inner dim
CHUNK = 512    # query-chunk width (moving free dim)

_cache = {}


def _build(nq=NQ, nk=NK):
    import concourse.tile as tile
    from concourse import bacc, mybir

    F32 = mybir.dt.float32
    F32R = mybir.dt.float32r
    BF16 = mybir.dt.bfloat16
    Exp = mybir.ActivationFunctionType.Exp

    n_chunks = nq // CHUNK
    m_tiles = nk // P        # key tiles of 128
    e_tiles = EI // P        # 8
    d_tiles = D // P         # 4
    m_chunks = nk // CHUNK   # phase-A key chunks

    nc = bacc.Bacc("TRN2", target_bir_lowering=False, debug=False)

    S1T = nc.dram_tensor("S1T", [D, nk], F32R, kind="ExternalInput").ap()
    S2T = nc.dram_tensor("S2T", [D, nq], F32R, kind="ExternalInput").ap()
    Wq = nc.dram_tensor("Wq", [D, EI], F32R, kind="ExternalInput").ap()
    Wk = nc.dram_tensor("Wk", [D, EI], F32R, kind="ExternalInput").ap()
    Wv = nc.dram_tensor("Wv", [D, EI], F32R, kind="ExternalInput").ap()
    Wo = nc.dram_tensor("Wo", [EI, D], F32, kind="ExternalInput").ap()
    BO = nc.dram_tensor("BO", [P, d_tiles], F32, kind="ExternalInput").ap()
    OUT = nc.dram_tensor("OUT", [D, nq], F32, kind="ExternalOutput").ap()

    with tile.TileContext(nc) as tc, ExitStack() as ctx, \
            nc.allow_low_precision(reason="float32r staging for matmul operands"):
        const = ctx.enter_context(tc.tile_pool(name="const", bufs=1))
        w_pool = ctx.enter_context(tc.tile_pool(name="w_pool", bufs=1))
        dram = ctx.enter_context(tc.tile_pool(name="dram", bufs=1, space="DRAM"))
        ps_mm = ctx.enter_context(tc.tile_pool(name="ps_mm", bufs=3, space="PSUM"))
        ps_ut = ctx.enter_context(tc.tile_pool(name="ps_ut", bufs=4, space="PSUM"))
        ps_sum = ctx.enter_context(tc.tile_pool(name="ps_sum", bufs=1, space="PSUM"))

        # constants
        ones_f = const.tile([P, 1], F32, name="ones_f")
        nc.any.memset(ones_f[:], 1.0)
        ones_col = const.tile([P, 1], BF16, name="ones_col")
        nc.vector.tensor_copy(ones_col[:], ones_f[:])
        bo_t = const.tile([P, d_tiles], F32, name="bo_t")
        nc.sync.dma_start(bo_t[:], BO[:, :])

        # persistent weights: Wq as [p, d_tile, e], Wo as [p, e_tile, d]
        # (DMAs are emitted inside phase A, after the phase-A critical loads)
        wq_t = w_pool.tile([P, d_tiles, EI], F32R, name="wq_t")
        wo_t = w_pool.tile([P, e_tiles, D], BF16, name="wo_t")
        kt_res = w_pool.tile([P, e_tiles, nk], F32R, name="kt_res")

        # DRAM scratch for V (K^T stays SBUF-resident)
        v_d = dram.tile([m_tiles, P, EI], BF16, name="v_d")

        # ---------------- Phase A: KT and V ----------------
        with tc.tile_pool(name="pa_w", bufs=1) as pa_w, \
                tc.tile_pool(name="s1_pool", bufs=3) as s1_pool, \
                tc.tile_pool(name="evA", bufs=4) as evA, \
                nc.named_scope("phaseA"):
            wk_t = pa_w.tile([P, d_tiles, EI], F32R, name="wk_t")
            wk_r = Wk.rearrange("(t p) e -> p t e", p=P)
            wv_t = pa_w.tile([P, d_tiles, EI], F32R, name="wv_t")
            wv_r = Wv.rearrange("(t p) e -> p t e", p=P)

            s1_tiles = []
            s1_r = [
                S1T[:, mc * CHUNK:(mc + 1) * CHUNK].rearrange(
                    "(t p) m -> p t m", p=P)
                for mc in range(m_chunks)
            ]
            # chunk 0: interleave wk / s1 slices per d-tile so the first
            # accumulation group's operands arrive first
            s1_0 = s1_pool.tile([P, d_tiles, CHUNK], F32R, name="s1_t", tag="s1")
            s1_tiles.append(s1_0)
            for dt_ in range(d_tiles):
                nc.sync.dma_start(wk_t[:, dt_, :], wk_r[:, dt_, :])
                nc.sync.dma_start(s1_0[:, dt_, :], s1_r[0][:, dt_, :])
            for mc in range(1, m_chunks):
                s1_t = s1_pool.tile([P, d_tiles, CHUNK], F32R, name="s1_t", tag="s1")
                nc.sync.dma_start(s1_t[:], s1_r[mc])
                s1_tiles.append(s1_t)
                if mc == 1:
                    for dt_ in range(d_tiles):
                        nc.sync.dma_start(wv_t[:, dt_, :], wv_r[:, dt_, :])

            wq_r = Wq.rearrange("(t p) e -> p t e", p=P)
            for dt_ in range(d_tiles):
                nc.sync.dma_start(wq_t[:, dt_, :], wq_r[:, dt_, :])
            wo_r = Wo.rearrange("(t p) d -> p t d", p=P)
            for et_ in range(e_tiles):
                nc.gpsimd.dma_start(wo_t[:, et_, :], wo_r[:, et_, :])

            for mc in range(m_chunks):
                s1_t = s1_tiles[mc]
                # KT for m-chunk 0 first (unblocks chunk-0 scoresT), then V
                # before KT for later chunks (V feeds chunk-0 UT earlier)
                def _emit_kt(mc, s1_t):
                    for et in range(e_tiles):
                        acc = ps_mm.tile([P, CHUNK], F32, name="accA", tag="mm")
                        for dt_ in range(d_tiles):
                            nc.tensor.matmul(
                                acc[:],
                                wk_t[:, dt_, et * P:(et + 1) * P],
                                s1_t[:, dt_, :],
                                start=(dt_ == 0), stop=(dt_ == d_tiles - 1),
                            )
                        nc.vector.tensor_copy(
                            kt_res[:, et, mc * CHUNK:(mc + 1) * CHUNK], acc[:])

                def _emit_v(mc, s1_t):
                    for ml in range(CHUNK // P):
                        mt = mc * (CHUNK // P) + ml
                        for ec in range(EI // CHUNK):
                            accv = ps_mm.tile([P, CHUNK], F32, name="accV", tag="mm")
                            for dt_ in range(d_tiles):
                                nc.tensor.matmul(
                                    accv[:],
                                    s1_t[:, dt_, ml * P:(ml + 1) * P],
                                    wv_t[:, dt_, ec * CHUNK:(ec + 1) * CHUNK],
                                    start=(dt_ == 0), stop=(dt_ == d_tiles - 1),
                                )
                            evv = evA.tile([P, CHUNK], BF16, name="evV", tag="evV")
                            nc.vector.tensor_copy(evv[:], accv[:])
                            nc.sync.dma_start(
                                v_d[mt, :, ec * CHUNK:(ec + 1) * CHUNK], evv[:])

                if mc == 0:
                    _emit_kt(mc, s1_t)
                    _emit_v(mc, s1_t)
                else:
                    _emit_v(mc, s1_t)
                    _emit_kt(mc, s1_t)

        # ---------------- Phase B: attention ----------------
        s2_pool = ctx.enter_context(tc.tile_pool(name="s2_pool", bufs=2))
        qt_pool = ctx.enter_context(tc.tile_pool(name="qt_pool", bufs=2))
        e_pool = ctx.enter_context(tc.tile_pool(name="e_pool", bufs=m_tiles + 4))
        v_pool = ctx.enter_context(tc.tile_pool(name="v_pool", bufs=6))
        mk_pool = ctx.enter_context(tc.tile_pool(name="mk_pool", bufs=e_tiles + 2))
        out_pool = ctx.enter_context(tc.tile_pool(name="out_pool", bufs=2))
        misc = ctx.enter_context(tc.tile_pool(name="misc", bufs=2))

        for c in range(n_chunks):
          with nc.named_scope(f"chunk{c}"):
            csl = slice(c * CHUNK, (c + 1) * CHUNK)
            s2_t = s2_pool.tile([P, d_tiles, CHUNK], F32R, name="s2_t", tag="s2")
            nc.sync.dma_start(
                s2_t[:], S2T[:, csl].rearrange("(t p) n -> p t n", p=P))

            # QT chunk [e_tile, 128, CHUNK]
            qt_t = qt_pool.tile([P, e_tiles, CHUNK], F32R, name="qt_t", tag="qt")
            for et in range(e_tiles):
                accq = ps_mm.tile([P, CHUNK], F32, name="accQ", tag="mm")
                for dt_ in range(d_tiles):
                    nc.tensor.matmul(
                        accq[:],
                        wq_t[:, dt_, et * P:(et + 1) * P],
                        s2_t[:, dt_, :],
                        start=(dt_ == 0), stop=(dt_ == d_tiles - 1),
                    )
                nc.vector.tensor_copy(qt_t[:, et, :], accq[:])

            # scoresT tiles + exp + running column sums
            sum_ps = ps_sum.tile([1, CHUNK], F32, name="sum_ps", tag="sum")
            e_list = []
            for mt in range(m_tiles):
                acc_s = ps_mm.tile([P, CHUNK], F32, name="acc_s", tag="mm")
                for et in range(e_tiles):
                    nc.tensor.matmul(
                        acc_s[:],
                        kt_res[:, et, mt * P:(mt + 1) * P],
                        qt_t[:, et, :],
                        start=(et == 0), stop=(et == e_tiles - 1),
                    )
                e_t = e_pool.tile([P, CHUNK], BF16, name="e_t", tag="e")
                nc.scalar.activation(e_t[:], acc_s[:], Exp)
                e_list.append(e_t)
                nc.tensor.matmul(
                    sum_ps[:], ones_col[:], e_t[:],
                    start=(mt == 0), stop=(mt == m_tiles - 1),
                )

            # 1/sumexp broadcast to all partitions
            sum_sb = misc.tile([1, CHUNK], F32, name="sum_sb", tag="sumsb")
            nc.vector.tensor_copy(sum_sb[:], sum_ps[:])
            recip = misc.tile([1, CHUNK], F32, name="recip", tag="recip")
            nc.vector.reciprocal(recip[:], sum_sb[:])
            bc = misc.tile([P, CHUNK], F32, name="bc", tag="bc")
            nc.gpsimd.partition_broadcast(bc[:], recip[:])

            # UT = V^T @ E in two 4-bank passes; normalize on eviction
            masked = []
            for half in range(2):
                ut_list = [
                    ps_ut.tile([P, CHUNK], F32, name="ut", tag="ut")
                    for _ in range(4)
                ]
                for mt in range(m_tiles):
                    v_t = v_pool.tile([P, CHUNK], BF16, name="v_t", tag="v")
                    nc.sync.dma_start(
                        v_t[:], v_d[mt, :, half * CHUNK:(half + 1) * CHUNK])
                    for ei in range(4):
                        nc.tensor.matmul(
                            ut_list[ei][:],
                            v_t[:, ei * P:(ei + 1) * P],
                            e_list[mt][:],
                            start=(mt == 0), stop=(mt == m_tiles - 1),
                        )
                for ei in range(4):
                    m_t = mk_pool.tile([P, CHUNK], BF16, name="m_t", tag="mk")
                    nc.vector.tensor_mul(m_t[:], ut_list[ei][:], bc[:])
                    masked.append(m_t)

            # outT = Wo^T @ maskedT + bo
            for dt_ in range(d_tiles):
                acc_o = ps_mm.tile([P, CHUNK], F32, name="acc_o", tag="mm")
                for et in range(e_tiles):
                    nc.tensor.matmul(
                        acc_o[:],
                        wo_t[:, et, dt_ * P:(dt_ + 1) * P],
                        masked[et][:],
                        start=(et == 0), stop=(et == e_tiles - 1),
                    )
                o_sb = out_pool.tile([P, CHUNK], F32, name="o_sb", tag="osb")
                nc.vector.tensor_scalar_add(o_sb[:], acc_o[:], bo_t[:, dt_:dt_ + 1])
                nc.sync.dma_start(OUT[dt_ * P:(dt_ + 1) * P, csl], o_sb[:])

    nc.compile()
    return nc


def _get_nc(nq=NQ, nk=NK):
    key = (nq, nk)
    if key not in _cache:
        _cache[key] = _build(nq, nk)
    return _cache[key]


def kernel(S1, S2, Wq, Wk, Wv, Wo, bo, _trace=False):
    from concourse.bass_utils import run_bass_kernel_spmd

    S1 = np.asarray(S1, np.float32)
    S2 = np.asarray(S2, np.float32)
    b, nk, _ = S1.shape
    _, nq, _ = S2.shape
    nc = _get_nc(nq, nk)

    bo_r = np.ascontiguousarray(
        np.asarray(bo, np.float32).reshape(D // P, P).T)  # [128, d_tiles]
    wq = np.ascontiguousarray(np.asarray(Wq, np.float32))
    wk = np.ascontiguousarray(np.asarray(Wk, np.float32))
    wv = np.ascontiguousarray(np.asarray(Wv, np.float32))
    wo = np.ascontiguousarray(np.asarray(Wo, np.float32))

    in_maps = []
    for i in range(b):
        in_maps.append({
            "S1T": np.ascontiguousarray(S1[i].T),
            "S2T": np.ascontiguousarray(S2[i].T),
            "Wq": wq, "Wk": wk, "Wv": wv, "Wo": wo, "BO": bo_r,
        })

    res = run_bass_kernel_spmd(nc, in_maps, list(range(b)), trace=_trace)
    out = np.stack([np.asarray(res.results[i]["OUT"]).T for i in range(b)])
    if _trace:
        kernel.last_result = res
    return np.ascontiguousarray(out.astype(np.float32))



# revision 2
# speedup vs baseline: 1.9226x; 1.9226x over previous
"""Cross-attention Trainium2 kernel (Bass/Tile), data-parallel over batch on 8 cores.

Reference computation per batch element b (no 1/sqrt(d) scaling):
    Q = S2[b] @ Wq            [N2, E]
    K = S1[b] @ Wk            [N1, E]
    V = S1[b] @ Wv            [N1, E]
    A = softmax(Q @ K^T, -1)  [N2, N1]
    out[b] = (A @ V) @ Wo + bo  [N2, D]

Low-rank reformulation (E = 1024 > D = 512, so fold the weight pairs):
    W1 = Wq @ Wk^T  [D, D]    (folded on host, f64 accumulation)
    W2 = Wv @ Wo    [D, D]
    scores = S2 (Wq Wk^T) S1^T = (S2 @ W1) @ S1^T     -- contraction D, not E
    out    = A S1 (Wv Wo) + bo = (A @ S1) @ W2 + bo   -- contraction D, not E
This halves the two big matmuls' contraction depth and removes the Q/K/V
projections entirely: ~393K PE cycles/core vs ~819K for the direct form.

Device layout is fully transposed (feature dims on SBUF partitions):
    host supplies S1T = S1[b].T, S2T = S2[b].T  [D, N], S1B = S1[b] in bf16
    per 512-query chunk:
      TT = W1^T-blocks @ S2T chunk         [d2, n]   (f32r)
      scoresT tiles [m-part, n-free] -> exp (no max subtraction: |score| <=
      ~70 and exp stays in fp32 range) -> ones-matmul row sums in PSUM ->
      reciprocal -> gpsimd partition_broadcast -> ZT = S1B^T-blocks @ E
      accumulated in 4 PSUM banks, normalized during eviction ->
      outT = W2-blocks @ ZT + bo -> DRAM [D, N2]; host transposes back.

Matmul operands are float32r on the score/output path (TF32-like 12-bit
mantissa, full PE rate at moving dim >= 256) and bf16 on the A@S1 path.
"""
import sys

sys.path.insert(0, "/opt/trn_rl_repo")

import numpy as np
import ml_dtypes
from contextlib import ExitStack

P = 128
N_CORES = 8
B = 8          # batch (one element per core)
NQ = 2048      # queries (N2)
NK = 2048      # keys (N1)
D = 512        # query/cross dim
CHUNK = 512    # query-chunk width (moving free dim)

_cache = {}


def _build(nq=NQ, nk=NK):
    import concourse.tile as tile
    from concourse import bacc, mybir

    F32 = mybir.dt.float32
    F32R = mybir.dt.float32r
    BF16 = mybir.dt.bfloat16
    Exp = mybir.ActivationFunctionType.Exp

    n_chunks = nq // CHUNK
    m_tiles = nk // P        # key tiles of 128 (16)
    d_tiles = D // P         # 4
    m_chunks = nk // CHUNK   # S1T load chunks (4)

    nc = bacc.Bacc("TRN2", target_bir_lowering=False, debug=False)

    S1T = nc.dram_tensor("S1T", [D, nk], F32R, kind="ExternalInput").ap()
    S1B = nc.dram_tensor("S1B", [nk, D], BF16, kind="ExternalInput").ap()
    S2T = nc.dram_tensor("S2T", [D, nq], F32R, kind="ExternalInput").ap()
    W1 = nc.dram_tensor("W1", [D, D], F32R, kind="ExternalInput").ap()
    W2 = nc.dram_tensor("W2", [D, D], F32R, kind="ExternalInput").ap()
    BO = nc.dram_tensor("BO", [P, d_tiles], F32, kind="ExternalInput").ap()
    OUT = nc.dram_tensor("OUT", [D, nq], F32, kind="ExternalOutput").ap()

    with tile.TileContext(nc) as tc, ExitStack() as ctx, \
            nc.allow_low_precision(reason="float32r/bf16 staging for matmul operands"):
        const = ctx.enter_context(tc.tile_pool(name="const", bufs=1))
        w_pool = ctx.enter_context(tc.tile_pool(name="w_pool", bufs=1))
        s2_pool = ctx.enter_context(tc.tile_pool(name="s2_pool", bufs=2))
        tt_pool = ctx.enter_context(tc.tile_pool(name="tt_pool", bufs=2))
        e_pool = ctx.enter_context(tc.tile_pool(name="e_pool", bufs=m_tiles + 2))
        zt_pool = ctx.enter_context(tc.tile_pool(name="zt_pool", bufs=2))
        out_pool = ctx.enter_context(tc.tile_pool(name="out_pool", bufs=2))
        misc = ctx.enter_context(tc.tile_pool(name="misc", bufs=2))
        ps_mm = ctx.enter_context(tc.tile_pool(name="ps_mm", bufs=3, space="PSUM"))
        ps_z = ctx.enter_context(tc.tile_pool(name="ps_z", bufs=4, space="PSUM"))
        ps_sum = ctx.enter_context(tc.tile_pool(name="ps_sum", bufs=1, space="PSUM"))

        # constants
        ones_f = const.tile([P, 1], F32, name="ones_f")
        nc.any.memset(ones_f[:], 1.0)
        ones_col = const.tile([P, 1], BF16, name="ones_col")
        nc.vector.tensor_copy(ones_col[:], ones_f[:])
        bo_t = const.tile([P, d_tiles], F32, name="bo_t")

        # persistent tensors
        w1_t = w_pool.tile([P, d_tiles, D], F32R, name="w1_t")
        w2_t = w_pool.tile([P, d_tiles, D], F32R, name="w2_t")
        s1t_res = w_pool.tile([P, d_tiles, nk], F32R, name="s1t_res")
        s1b_res = w_pool.tile([P, m_tiles, D], BF16, name="s1b_res")

        # --- DMA priority order on the sync queue (critical path first):
        # W1, S2 chunk 0, S1T (split by key chunk) ---
        nc.sync.dma_start(w1_t[:], W1.rearrange("(t p) e -> p t e", p=P))
        s2_tiles = []
        s2_0 = s2_pool.tile([P, d_tiles, CHUNK], F32R, name="s2_t", tag="s2")
        nc.sync.dma_start(
            s2_0[:], S2T[:, 0:CHUNK].rearrange("(t p) n -> p t n", p=P))
        s2_tiles.append(s2_0)
        for mc in range(m_chunks):
            nc.sync.dma_start(
                s1t_res[:, :, mc * CHUNK:(mc + 1) * CHUNK],
                S1T[:, mc * CHUNK:(mc + 1) * CHUNK].rearrange(
                    "(t p) m -> p t m", p=P))
        # non-critical loads on the gpsimd queue (parallel with sync queue)
        nc.gpsimd.dma_start(
            s1b_res[:], S1B.rearrange("(t p) d -> p t d", p=P))
        nc.gpsimd.dma_start(w2_t[:], W2.rearrange("(t p) e -> p t e", p=P))
        nc.gpsimd.dma_start(bo_t[:], BO[:, :])

        def emit_tt(c, s2_t):
            """TT[d2, n] = sum_d1 W1[d1, d2] S2T[d1, n] for chunk c."""
            tt_t = tt_pool.tile([P, d_tiles, CHUNK], F32R, name="tt_t", tag="tt")
            for d2t in range(d_tiles):
                acc = ps_mm.tile([P, CHUNK], F32, name="accT", tag="mm")
                for d1t in range(d_tiles):
                    nc.tensor.matmul(
                        acc[:],
                        w1_t[:, d1t, d2t * P:(d2t + 1) * P],
                        s2_t[:, d1t, :],
                        start=(d1t == 0), stop=(d1t == d_tiles - 1),
                    )
                nc.vector.tensor_copy(tt_t[:, d2t, :], acc[:])
            return tt_t

        def emit_out(c, zt_t):
            """outT[do, n] = sum_dz W2[dz, do] ZT[dz, n] + bo for chunk c."""
            csl = slice(c * CHUNK, (c + 1) * CHUNK)
            for dot in range(d_tiles):
                acc_o = ps_mm.tile([P, CHUNK], F32, name="acc_o", tag="mm")
                for dzt in range(d_tiles):
                    nc.tensor.matmul(
                        acc_o[:],
                        w2_t[:, dzt, dot * P:(dot + 1) * P],
                        zt_t[:, dzt, :],
                        start=(dzt == 0), stop=(dzt == d_tiles - 1),
                    )
                o_sb = out_pool.tile([P, CHUNK], F32, name="o_sb", tag="osb")
                nc.vector.tensor_scalar_add(o_sb[:], acc_o[:], bo_t[:, dot:dot + 1])
                nc.sync.dma_start(OUT[dot * P:(dot + 1) * P, csl], o_sb[:])

        prev_zt = None
        prev_c = -1
        tt_t = emit_tt(0, s2_0)
        for c in range(n_chunks):
          with nc.named_scope(f"chunk{c}"):
            # prefetch next chunk's S2 (s2_pool bufs=2 -> one ahead)
            if c + 1 < n_chunks:
                csl_n = slice((c + 1) * CHUNK, (c + 2) * CHUNK)
                s2_n = s2_pool.tile([P, d_tiles, CHUNK], F32R, name="s2_t", tag="s2")
                nc.sync.dma_start(
                    s2_n[:], S2T[:, csl_n].rearrange("(t p) n -> p t n", p=P))
                s2_tiles.append(s2_n)

            # scoresT tiles + exp + running column sums
            sum_ps = ps_sum.tile([1, CHUNK], F32, name="sum_ps", tag="sum")
            e_list = []
            for mt in range(m_tiles):
                acc_s = ps_mm.tile([P, CHUNK], F32, name="acc_s", tag="mm")
                for d2t in range(d_tiles):
                    nc.tensor.matmul(
                        acc_s[:],
                        s1t_res[:, d2t, mt * P:(mt + 1) * P],
                        tt_t[:, d2t, :],
                        start=(d2t == 0), stop=(d2t == d_tiles - 1),
                    )
                e_t = e_pool.tile([P, CHUNK], BF16, name="e_t", tag="e")
                nc.scalar.activation(e_t[:], acc_s[:], Exp)
                e_list.append(e_t)
                nc.tensor.matmul(
                    sum_ps[:], ones_col[:], e_t[:],
                    start=(mt == 0), stop=(mt == m_tiles - 1),
                )

            # 1/sumexp broadcast to all partitions
            sum_sb = misc.tile([1, CHUNK], F32, name="sum_sb", tag="sumsb")
            nc.vector.tensor_copy(sum_sb[:], sum_ps[:])
            recip = misc.tile([1, CHUNK], F32, name="recip", tag="recip")
            nc.vector.reciprocal(recip[:], sum_sb[:])
            bc = misc.tile([P, CHUNK], F32, name="bc", tag="bc")
            nc.gpsimd.partition_broadcast(bc[:], recip[:])

            # ZT = S1^T @ E accumulated in 4 PSUM banks over all m tiles
            z_list = [
                ps_z.tile([P, CHUNK], F32, name="zt_ps", tag="z")
                for _ in range(d_tiles)
            ]
            for mt in range(m_tiles):
                for dt_ in range(d_tiles):
                    nc.tensor.matmul(
                        z_list[dt_][:],
                        s1b_res[:, mt, dt_ * P:(dt_ + 1) * P],
                        e_list[mt][:],
                        start=(mt == 0), stop=(mt == m_tiles - 1),
                    )

            # PE: next chunk's TT while ZT is evicted / normalized on DVE
            this_tt = tt_t
            if c + 1 < n_chunks:
                tt_t = emit_tt(c + 1, s2_tiles[c + 1])

            zt_t = zt_pool.tile([P, d_tiles, CHUNK], F32R, name="zt_t", tag="zt")
            for dt_ in range(d_tiles):
                nc.vector.tensor_mul(zt_t[:, dt_, :], z_list[dt_][:], bc[:])

            # previous chunk's output projection was already emitted; emit ours
            emit_out(c, zt_t)

    nc.compile()
    return nc


def _get_nc(nq=NQ, nk=NK):
    key = (nq, nk)
    if key not in _cache:
        _cache[key] = _build(nq, nk)
    return _cache[key]


def kernel(S1, S2, Wq, Wk, Wv, Wo, bo, _trace=False):
    from concourse.bass_utils import run_bass_kernel_spmd

    S1 = np.asarray(S1, np.float32)
    S2 = np.asarray(S2, np.float32)
    b, nk, _ = S1.shape
    _, nq, _ = S2.shape
    nc = _get_nc(nq, nk)

    # Fold weight pairs on host (f64 accumulation for accuracy)
    w1 = np.ascontiguousarray(
        (np.asarray(Wq, np.float64) @ np.asarray(Wk, np.float64).T)
        .astype(np.float32))
    w2 = np.ascontiguousarray(
        (np.asarray(Wv, np.float64) @ np.asarray(Wo, np.float64))
        .astype(np.float32))
    bo_r = np.ascontiguousarray(
        np.asarray(bo, np.float32).reshape(D // P, P).T)  # [128, d_tiles]

    in_maps = []
    for i in range(b):
        in_maps.append({
            "S1T": np.ascontiguousarray(S1[i].T),
            "S1B": np.ascontiguousarray(S1[i].astype(ml_dtypes.bfloat16)),
            "S2T": np.ascontiguousarray(S2[i].T),
            "W1": w1, "W2": w2, "BO": bo_r,
        })

    res = run_bass_kernel_spmd(nc, in_maps, list(range(b)), trace=_trace)
    out = np.stack([np.asarray(res.results[i]["OUT"]).T for i in range(b)])
    if _trace:
        kernel.last_result = res
    return np.ascontiguousarray(out.astype(np.float32))


# revision 3
# speedup vs baseline: 2.0299x; 1.0558x over previous
"""Cross-attention Trainium2 kernel (Bass/Tile), data-parallel over batch on 8 cores.

Reference computation per batch element b (no 1/sqrt(d) scaling):
    Q = S2[b] @ Wq; K = S1[b] @ Wk; V = S1[b] @ Wv
    A = softmax(Q @ K^T, -1)
    out[b] = (A @ V) @ Wo + bo

Low-rank reformulation (inner E = 1024 > D = 512, so fold the weight pairs):
    W1 = Wq @ Wk^T  [D, D]   (folded on host, f64 accumulation)
    W2 = Wv @ Wo    [D, D]
    scores = (S2 @ W1) @ S1^T     -- contraction D=512, not E=1024
    out    = (A @ S1) @ W2 + bo   -- contraction D=512, not E=1024
This halves the two big matmuls' contraction depth and removes the Q/K/V
projections entirely: ~393K PE cycles/core vs ~819K for the direct form.

Device layout is fully transposed (feature dims on SBUF partitions). All
inputs arrive pre-rearranged from the host in exactly the SBUF tile layout
([128 partitions, ...] dense per partition) so every DMA is a contiguous
per-partition copy (cheap descriptors, no HWDGE serialization).

Per 512-query chunk:
    TT = W1-blocks^T @ S2T chunk          [d2, n]  (f32r)
    scoresT tiles [m-part, n-free] -> exp (no max subtraction: |score| <=
    ~70, exp stays in fp32 range) -> ones-matmul row sums in PSUM ->
    reciprocal -> gpsimd partition_broadcast -> ZT = S1-blocks^T @ E
    accumulated in 4 PSUM banks over all m tiles, normalized during
    eviction -> outT = W2-blocks^T @ ZT + bo -> DRAM [D, N2]; host
    transposes back.

Matmul operands are float32r on the score/output path (TF32-like 12-bit
mantissa, full PE rate at moving dim >= 256) and bf16 on the A@S1 path.
"""
import sys

sys.path.insert(0, "/opt/trn_rl_repo")

import numpy as np
import ml_dtypes
from contextlib import ExitStack

P = 128
N_CORES = 8
B = 8          # batch (one element per core)
NQ = 2048      # queries (N2)
NK = 2048      # keys (N1)
D = 512        # query/cross dim
CHUNK = 512    # query-chunk width (moving free dim)

_cache = {}


def _build(nq=NQ, nk=NK):
    import concourse.tile as tile
    from concourse import bacc, mybir

    F32 = mybir.dt.float32
    F32R = mybir.dt.float32r
    BF16 = mybir.dt.bfloat16
    Exp = mybir.ActivationFunctionType.Exp

    n_chunks = nq // CHUNK
    m_tiles = nk // P        # key tiles of 128 (16)
    d_tiles = D // P         # 4
    m_chunks = nk // CHUNK   # S1T load quarters (4)

    nc = bacc.Bacc("TRN2", target_bir_lowering=False, debug=False)

    # all dram tensors already in SBUF tile layout (dense per partition)
    S1T = nc.dram_tensor("S1T", [P, d_tiles, nk], F32R, kind="ExternalInput").ap()
    S1B = nc.dram_tensor("S1B", [P, m_tiles, D], BF16, kind="ExternalInput").ap()
    S2T = nc.dram_tensor("S2T", [P, d_tiles, nq], F32R, kind="ExternalInput").ap()
    W1 = nc.dram_tensor("W1", [P, d_tiles, D], F32R, kind="ExternalInput").ap()
    W2 = nc.dram_tensor("W2", [P, d_tiles, D], F32R, kind="ExternalInput").ap()
    BO = nc.dram_tensor("BO", [P, d_tiles], F32, kind="ExternalInput").ap()
    OUT = nc.dram_tensor("OUT", [D, nq], F32, kind="ExternalOutput").ap()

    with tile.TileContext(nc) as tc, ExitStack() as ctx, \
            nc.allow_low_precision(reason="float32r/bf16 staging for matmul operands"):
        const = ctx.enter_context(tc.tile_pool(name="const", bufs=1))
        w_pool = ctx.enter_context(tc.tile_pool(name="w_pool", bufs=1))
        tt_pool = ctx.enter_context(tc.tile_pool(name="tt_pool", bufs=2))
        e_pool = ctx.enter_context(tc.tile_pool(name="e_pool", bufs=m_tiles + 2))
        zt_pool = ctx.enter_context(tc.tile_pool(name="zt_pool", bufs=2))
        out_pool = ctx.enter_context(tc.tile_pool(name="out_pool", bufs=2))
        misc = ctx.enter_context(tc.tile_pool(name="misc", bufs=2))
        ps_mm = ctx.enter_context(tc.tile_pool(name="ps_mm", bufs=3, space="PSUM"))
        ps_z = ctx.enter_context(tc.tile_pool(name="ps_z", bufs=4, space="PSUM"))
        ps_sum = ctx.enter_context(tc.tile_pool(name="ps_sum", bufs=1, space="PSUM"))

        # constants
        ones_f = const.tile([P, 1], F32, name="ones_f")
        nc.any.memset(ones_f[:], 1.0)
        ones_col = const.tile([P, 1], BF16, name="ones_col")
        nc.vector.tensor_copy(ones_col[:], ones_f[:])
        bo_t = const.tile([P, d_tiles], F32, name="bo_t")

        # persistent tensors
        w1_t = w_pool.tile([P, d_tiles, D], F32R, name="w1_t")
        w2_t = w_pool.tile([P, d_tiles, D], F32R, name="w2_t")
        s1t_res = w_pool.tile([P, d_tiles, nk], F32R, name="s1t_res")
        s1b_res = w_pool.tile([P, m_tiles, D], BF16, name="s1b_res")
        s2_res = w_pool.tile([P, d_tiles, nq], F32R, name="s2_res")

        # --- startup DMA: two HWDGE rings in parallel, critical path first.
        # sync ring:   W1, S1T quarters (gates scores chunk 0)
        # scalar ring: S2 chunk 0 (gates TT chunk 0), then the rest
        nc.sync.dma_start(w1_t[:], W1[:, :, :])
        nc.scalar.dma_start(
            s2_res[:, :, 0:CHUNK], S2T[:, :, 0:CHUNK])
        for mc in range(m_chunks):
            sl = slice(mc * CHUNK, (mc + 1) * CHUNK)
            nc.sync.dma_start(s1t_res[:, :, sl], S1T[:, :, sl])
        for mh in range(2):
            sl = slice(mh * (m_tiles // 2), (mh + 1) * (m_tiles // 2))
            nc.scalar.dma_start(s1b_res[:, sl, :], S1B[:, sl, :])
        nc.scalar.dma_start(w2_t[:], W2[:, :, :])
        nc.scalar.dma_start(bo_t[:], BO[:, :])
        for c in range(1, n_chunks):
            sl = slice(c * CHUNK, (c + 1) * CHUNK)
            nc.sync.dma_start(s2_res[:, :, sl], S2T[:, :, sl])

        def emit_tt(c):
            """TT[d2, n] = sum_d1 W1[d1, d2] S2T[d1, n] for chunk c."""
            csl = slice(c * CHUNK, (c + 1) * CHUNK)
            tt_t = tt_pool.tile([P, d_tiles, CHUNK], F32R, name="tt_t", tag="tt")
            for d2t in range(d_tiles):
                acc = ps_mm.tile([P, CHUNK], F32, name="accT", tag="mm")
                for d1t in range(d_tiles):
                    nc.tensor.matmul(
                        acc[:],
                        w1_t[:, d1t, d2t * P:(d2t + 1) * P],
                        s2_res[:, d1t, csl],
                        start=(d1t == 0), stop=(d1t == d_tiles - 1),
                    )
                nc.vector.tensor_copy(tt_t[:, d2t, :], acc[:])
            return tt_t

        def emit_out(c, zt_t):
            """outT[do, n] = sum_dz W2[dz, do] ZT[dz, n] + bo for chunk c."""
            csl = slice(c * CHUNK, (c + 1) * CHUNK)
            for dot in range(d_tiles):
                acc_o = ps_z.tile([P, CHUNK], F32, name="acc_o", tag="z")
                for dzt in range(d_tiles):
                    nc.tensor.matmul(
                        acc_o[:],
                        w2_t[:, dzt, dot * P:(dot + 1) * P],
                        zt_t[:, dzt, :],
                        start=(dzt == 0), stop=(dzt == d_tiles - 1),
                    )
                o_sb = out_pool.tile([P, CHUNK], F32, name="o_sb", tag="osb")
                nc.vector.tensor_scalar_add(o_sb[:], acc_o[:], bo_t[:, dot:dot + 1])
                eng = nc.sync if dot % 2 == 0 else nc.scalar
                eng.dma_start(OUT[dot * P:(dot + 1) * P, csl], o_sb[:])

        tt_t = emit_tt(0)
        for c in range(n_chunks):
          with nc.named_scope(f"chunk{c}"):
            # scoresT tiles + exp + running column sums
            sum_ps = ps_sum.tile([1, CHUNK], F32, name="sum_ps", tag="sum")
            e_list = []
            for mt in range(m_tiles):
                acc_s = ps_mm.tile([P, CHUNK], F32, name="acc_s", tag="mm")
                for d2t in range(d_tiles):
                    nc.tensor.matmul(
                        acc_s[:],
                        s1t_res[:, d2t, mt * P:(mt + 1) * P],
                        tt_t[:, d2t, :],
                        start=(d2t == 0), stop=(d2t == d_tiles - 1),
                    )
                e_t = e_pool.tile([P, CHUNK], BF16, name="e_t", tag="e")
                nc.scalar.activation(e_t[:], acc_s[:], Exp)
                e_list.append(e_t)
                nc.tensor.matmul(
                    sum_ps[:], ones_col[:], e_t[:],
                    start=(mt == 0), stop=(mt == m_tiles - 1),
                )

            # 1/sumexp broadcast to all partitions
            sum_sb = misc.tile([1, CHUNK], F32, name="sum_sb", tag="sumsb")
            nc.vector.tensor_copy(sum_sb[:], sum_ps[:])
            recip = misc.tile([1, CHUNK], F32, name="recip", tag="recip")
            nc.vector.reciprocal(recip[:], sum_sb[:])
            bc = misc.tile([P, CHUNK], F32, name="bc", tag="bc")
            nc.gpsimd.partition_broadcast(bc[:], recip[:])

            # ZT = S1^T @ E accumulated in 4 PSUM banks over all m tiles
            z_list = [
                ps_z.tile([P, CHUNK], F32, name="zt_ps", tag="z")
                for _ in range(d_tiles)
            ]
            for mt in range(m_tiles):
                for dt_ in range(d_tiles):
                    nc.tensor.matmul(
                        z_list[dt_][:],
                        s1b_res[:, mt, dt_ * P:(dt_ + 1) * P],
                        e_list[mt][:],
                        start=(mt == 0), stop=(mt == m_tiles - 1),
                    )

            # DVE: normalize ZT out of PSUM first (out-proj is gated on it) ...
            zt_t = zt_pool.tile([P, d_tiles, CHUNK], F32R, name="zt_t", tag="zt")
            for dt_ in range(d_tiles):
                nc.vector.tensor_mul(zt_t[:, dt_, :], z_list[dt_][:], bc[:])

            # ... while the PE runs the next chunk's TT, then our out-proj
            this_c = c
            if c + 1 < n_chunks:
                tt_t = emit_tt(c + 1)
            emit_out(this_c, zt_t)

    nc.compile()
    return nc


def _get_nc(nq=NQ, nk=NK):
    key = (nq, nk)
    if key not in _cache:
        _cache[key] = _build(nq, nk)
    return _cache[key]


def _tile_rows(a, t):
    """[t*128, X] row-major -> [128, t, X] (partition-major tile layout)."""
    x = a.shape[-1]
    return np.ascontiguousarray(a.reshape(t, P, x).transpose(1, 0, 2))


def kernel(S1, S2, Wq, Wk, Wv, Wo, bo, _trace=False):
    from concourse.bass_utils import run_bass_kernel_spmd

    S1 = np.asarray(S1, np.float32)
    S2 = np.asarray(S2, np.float32)
    b, nk, _ = S1.shape
    _, nq, _ = S2.shape
    nc = _get_nc(nq, nk)

    # Fold weight pairs on host (f64 accumulation for accuracy)
    w1 = (np.asarray(Wq, np.float64) @ np.asarray(Wk, np.float64).T
          ).astype(np.float32)
    w2 = (np.asarray(Wv, np.float64) @ np.asarray(Wo, np.float64)
          ).astype(np.float32)
    w1_r = _tile_rows(w1, D // P)
    w2_r = _tile_rows(w2, D // P)
    bo_r = np.ascontiguousarray(
        np.asarray(bo, np.float32).reshape(D // P, P).T)  # [128, d_tiles]

    in_maps = []
    for i in range(b):
        in_maps.append({
            "S1T": _tile_rows(np.ascontiguousarray(S1[i].T), D // P),
            "S1B": _tile_rows(S1[i].astype(ml_dtypes.bfloat16), nk // P),
            "S2T": _tile_rows(np.ascontiguousarray(S2[i].T), D // P),
            "W1": w1_r, "W2": w2_r, "BO": bo_r,
        })

    res = run_bass_kernel_spmd(nc, in_maps, list(range(b)), trace=_trace)
    out = np.stack([np.asarray(res.results[i]["OUT"]).T for i in range(b)])
    if _trace:
        kernel.last_result = res
    return np.ascontiguousarray(out.astype(np.float32))


# revision 4
# speedup vs baseline: 2.1963x; 1.0820x over previous
"""Cross-attention Trainium2 kernel (Bass/Tile), data-parallel over batch on 8 cores.

Reference computation per batch element b (no 1/sqrt(d) scaling):
    Q = S2[b] @ Wq; K = S1[b] @ Wk; V = S1[b] @ Wv
    A = softmax(Q @ K^T, -1)
    out[b] = (A @ V) @ Wo + bo

Low-rank reformulation (inner E = 1024 > D = 512, so fold the weight pairs):
    W1 = Wq @ Wk^T  [D, D]   (folded on host, f64 accumulation)
    W2 = Wv @ Wo    [D, D]
    scores = (S2 @ W1) @ S1^T     -- contraction D=512, not E=1024
    out    = (A @ S1) @ W2 + bo   -- contraction D=512, not E=1024
This halves the two big matmuls' contraction depth and removes the Q/K/V
projections entirely: ~380K PE cycles/core vs ~819K for the direct form.

Device layout is fully transposed (feature dims on SBUF partitions). All
inputs arrive pre-rearranged from the host in exactly the SBUF tile layout
([128 partitions, ...] dense per partition) so every DMA is a contiguous
per-partition copy. The scores chain (W1, S2T, TT, S1T) and output chain
(ZT, W2) run in fp16 (half the HBM bytes of f32, full PE rate, ~5e-4
element error); exp tiles stay bf16 for range (values up to e^61).

Per 512-query chunk:
    TT = W1-blocks^T @ S2T chunk          [d2, n]  (fp16)
    scoresT tiles [m-part, n-free] -> exp (no max subtraction: |score| <=
    ~70, exp stays in fp32 range) -> pairwise sum tree over the 16 exp
    tiles on the Pool engine -> single ones-matmul partition reduction ->
    reciprocal -> partition_broadcast -> ZT = S1-blocks^T @ E accumulated
    in 4 PSUM banks over all m tiles, normalized during eviction ->
    outT = W2-blocks^T @ ZT + bo -> DRAM [D, N2]; host transposes back.
"""
import sys

sys.path.insert(0, "/opt/trn_rl_repo")

import numpy as np
import ml_dtypes
from contextlib import ExitStack

P = 128
N_CORES = 8
B = 8          # batch (one element per core)
NQ = 2048      # queries (N2)
NK = 2048      # keys (N1)
D = 512        # query/cross dim
CHUNK = 512    # query-chunk width (moving free dim)

_cache = {}


def _build(nq=NQ, nk=NK):
    import concourse.tile as tile
    from concourse import bacc, mybir

    F32 = mybir.dt.float32
    F32R = mybir.dt.float32r
    F16 = mybir.dt.float16
    BF16 = mybir.dt.bfloat16
    Exp = mybir.ActivationFunctionType.Exp

    n_chunks = nq // CHUNK
    m_tiles = nk // P        # key tiles of 128 (16)
    d_tiles = D // P         # 4
    m_chunks = nk // CHUNK   # S1T load quarters (4)

    nc = bacc.Bacc("TRN2", target_bir_lowering=False, debug=False)

    # all dram tensors already in SBUF tile layout (dense per partition)
    S1T = nc.dram_tensor("S1T", [P, d_tiles, nk], F16, kind="ExternalInput").ap()
    S1B = nc.dram_tensor("S1B", [P, m_tiles, D], BF16, kind="ExternalInput").ap()
    S2T = nc.dram_tensor("S2T", [P, d_tiles, nq], F16, kind="ExternalInput").ap()
    W1 = nc.dram_tensor("W1", [P, d_tiles, D], F16, kind="ExternalInput").ap()
    W2 = nc.dram_tensor("W2", [P, d_tiles, D], F16, kind="ExternalInput").ap()
    BO = nc.dram_tensor("BO", [P, d_tiles], F32, kind="ExternalInput").ap()
    OUT = nc.dram_tensor("OUT", [D, nq], F32, kind="ExternalOutput").ap()

    with tile.TileContext(nc) as tc, ExitStack() as ctx, \
            nc.allow_low_precision(reason="fp16/bf16 staging for matmul operands"):
        const = ctx.enter_context(tc.tile_pool(name="const", bufs=1))
        w_pool = ctx.enter_context(tc.tile_pool(name="w_pool", bufs=1))
        tt_pool = ctx.enter_context(tc.tile_pool(name="tt_pool", bufs=2))
        e_pool = ctx.enter_context(tc.tile_pool(name="e_pool", bufs=m_tiles + 2))
        tree = ctx.enter_context(tc.tile_pool(name="tree", bufs=3))
        zt_pool = ctx.enter_context(tc.tile_pool(name="zt_pool", bufs=2))
        out_pool = ctx.enter_context(tc.tile_pool(name="out_pool", bufs=4))
        misc = ctx.enter_context(tc.tile_pool(name="misc", bufs=2))
        ps_mm = ctx.enter_context(tc.tile_pool(name="ps_mm", bufs=3, space="PSUM"))
        ps_z = ctx.enter_context(tc.tile_pool(name="ps_z", bufs=4, space="PSUM"))
        ps_sum = ctx.enter_context(tc.tile_pool(name="ps_sum", bufs=1, space="PSUM"))

        # constants
        ones_f = const.tile([P, 1], F32, name="ones_f")
        nc.any.memset(ones_f[:], 1.0)
        ones_r = const.tile([P, 1], F32R, name="ones_r")
        nc.vector.tensor_copy(ones_r[:], ones_f[:])
        bo_t = const.tile([P, d_tiles], F32, name="bo_t")

        # persistent tensors
        w1_t = w_pool.tile([P, d_tiles, D], F16, name="w1_t")
        w2_t = w_pool.tile([P, d_tiles, D], F16, name="w2_t")
        s1t_res = w_pool.tile([P, d_tiles, nk], F16, name="s1t_res")
        s1b_res = w_pool.tile([P, m_tiles, D], BF16, name="s1b_res")
        s2_res = w_pool.tile([P, d_tiles, nq], F16, name="s2_res")

        # --- startup DMA: two HWDGE rings + SWDGE, critical path first.
        # sync ring:   W1, S1T quarters (gates scores chunk 0), rest of S2
        # scalar ring: S2 chunk 0 (gates TT chunk 0)
        # pool SWDGE:  S1B, W2, BO (needed later)
        nc.sync.dma_start(w1_t[:], W1[:, :, :])
        nc.scalar.dma_start(s2_res[:, :, 0:CHUNK], S2T[:, :, 0:CHUNK])
        for mc in range(m_chunks):
            sl = slice(mc * CHUNK, (mc + 1) * CHUNK)
            nc.sync.dma_start(s1t_res[:, :, sl], S1T[:, :, sl])
        for mh in range(2):
            sl = slice(mh * (m_tiles // 2), (mh + 1) * (m_tiles // 2))
            nc.gpsimd.dma_start(s1b_res[:, sl, :], S1B[:, sl, :])
        nc.gpsimd.dma_start(w2_t[:], W2[:, :, :])
        nc.gpsimd.dma_start(bo_t[:], BO[:, :])
        for c in range(1, n_chunks):
            sl = slice(c * CHUNK, (c + 1) * CHUNK)
            nc.sync.dma_start(s2_res[:, :, sl], S2T[:, :, sl])

        def emit_tt(c):
            """TT[d2, n] = sum_d1 W1[d1, d2] S2T[d1, n] for chunk c."""
            csl = slice(c * CHUNK, (c + 1) * CHUNK)
            tt_t = tt_pool.tile([P, d_tiles, CHUNK], F16, name="tt_t", tag="tt")
            for d2t in range(d_tiles):
                acc = ps_mm.tile([P, CHUNK], F32, name="accT", tag="mm")
                for d1t in range(d_tiles):
                    nc.tensor.matmul(
                        acc[:],
                        w1_t[:, d1t, d2t * P:(d2t + 1) * P],
                        s2_res[:, d1t, csl],
                        start=(d1t == 0), stop=(d1t == d_tiles - 1),
                    )
                nc.vector.tensor_copy(tt_t[:, d2t, :], acc[:])
            return tt_t

        def emit_out(c, zt_t):
            """outT[do, n] = sum_dz W2[dz, do] ZT[dz, n] + bo for chunk c."""
            csl = slice(c * CHUNK, (c + 1) * CHUNK)
            for dot in range(d_tiles):
                acc_o = ps_z.tile([P, CHUNK], F32, name="acc_o", tag="z")
                for dzt in range(d_tiles):
                    nc.tensor.matmul(
                        acc_o[:],
                        w2_t[:, dzt, dot * P:(dot + 1) * P],
                        zt_t[:, dzt, :],
                        start=(dzt == 0), stop=(dzt == d_tiles - 1),
                    )
                o_sb = out_pool.tile([P, CHUNK], F32, name="o_sb", tag="osb")
                nc.vector.tensor_scalar_add(o_sb[:], acc_o[:], bo_t[:, dot:dot + 1])
                eng = nc.sync if dot % 2 == 0 else nc.scalar
                eng.dma_start(OUT[dot * P:(dot + 1) * P, csl], o_sb[:])

        tt_t = emit_tt(0)
        for c in range(n_chunks):
          with nc.named_scope(f"chunk{c}"):
            # scoresT tiles + exp; pairwise sum tree on the Pool engine
            e_list = []
            lvl1 = [None] * 8
            lvl2 = [None] * 4
            lvl3 = [None] * 2
            s_all = None
            for mt in range(m_tiles):
                acc_s = ps_mm.tile([P, CHUNK], F32, name="acc_s", tag="mm")
                for d2t in range(d_tiles):
                    nc.tensor.matmul(
                        acc_s[:],
                        s1t_res[:, d2t, mt * P:(mt + 1) * P],
                        tt_t[:, d2t, :],
                        start=(d2t == 0), stop=(d2t == d_tiles - 1),
                    )
                e_t = e_pool.tile([P, CHUNK], BF16, name="e_t", tag="e")
                nc.scalar.activation(e_t[:], acc_s[:], Exp)
                e_list.append(e_t)
                if mt % 2 == 1:
                    k = mt // 2
                    t1 = tree.tile([P, CHUNK], F32R, name="t1", tag="t1")
                    nc.gpsimd.tensor_add(t1[:], e_list[mt - 1][:], e_list[mt][:])
                    lvl1[k] = t1
                    if k % 2 == 1:
                        j = k // 2
                        t2 = tree.tile([P, CHUNK], F32R, name="t2", tag="t2")
                        nc.gpsimd.tensor_add(t2[:], lvl1[k - 1][:], lvl1[k][:])
                        lvl2[j] = t2
                        if j % 2 == 1:
                            i = j // 2
                            t3 = tree.tile([P, CHUNK], F32R, name="t3", tag="t3")
                            nc.gpsimd.tensor_add(t3[:], lvl2[j - 1][:], lvl2[j][:])
                            lvl3[i] = t3
                            if i == 1:
                                s_all = tree.tile(
                                    [P, CHUNK], F32R, name="t4", tag="t4")
                                nc.gpsimd.tensor_add(
                                    s_all[:], lvl3[0][:], lvl3[1][:])

            # ZT = S1^T @ E accumulated in 4 PSUM banks over all m tiles;
            # the single partition-reduction matmul for sumexp is slotted in
            # a few tiles into the loop (the Pool tree has finished by then)
            z_list = [
                ps_z.tile([P, CHUNK], F32, name="zt_ps", tag="z")
                for _ in range(d_tiles)
            ]
            sum_ps = None
            for mt in range(m_tiles):
                for dt_ in range(d_tiles):
                    nc.tensor.matmul(
                        z_list[dt_][:],
                        s1b_res[:, mt, dt_ * P:(dt_ + 1) * P],
                        e_list[mt][:],
                        start=(mt == 0), stop=(mt == m_tiles - 1),
                    )
                if mt == 3:
                    sum_ps = ps_sum.tile([1, CHUNK], F32, name="sum_ps", tag="sum")
                    nc.tensor.matmul(
                        sum_ps[:], ones_r[:], s_all[:], start=True, stop=True)
                    # 1/sumexp broadcast to all partitions
                    sum_sb = misc.tile([1, CHUNK], F32, name="sum_sb", tag="sumsb")
                    nc.vector.tensor_copy(sum_sb[:], sum_ps[:])
                    recip = misc.tile([1, CHUNK], F32, name="recip", tag="recip")
                    nc.vector.reciprocal(recip[:], sum_sb[:])
                    bc = misc.tile([P, CHUNK], F32, name="bc", tag="bc")
                    nc.gpsimd.partition_broadcast(bc[:], recip[:])

            # DVE: normalize ZT out of PSUM first (out-proj is gated on it) ...
            zt_t = zt_pool.tile([P, d_tiles, CHUNK], F16, name="zt_t", tag="zt")
            for dt_ in range(d_tiles):
                nc.vector.tensor_mul(zt_t[:, dt_, :], z_list[dt_][:], bc[:])

            # ... while the PE runs the next chunk's TT, then our out-proj
            this_c = c
            if c + 1 < n_chunks:
                tt_t = emit_tt(c + 1)
            emit_out(this_c, zt_t)

    nc.compile()
    return nc


def _get_nc(nq=NQ, nk=NK):
    key = (nq, nk)
    if key not in _cache:
        _cache[key] = _build(nq, nk)
    return _cache[key]


def _tile_rows(a, t):
    """[t*128, X] row-major -> [128, t, X] (partition-major tile layout)."""
    x = a.shape[-1]
    return np.ascontiguousarray(a.reshape(t, P, x).transpose(1, 0, 2))


def kernel(S1, S2, Wq, Wk, Wv, Wo, bo, _trace=False):
    from concourse.bass_utils import run_bass_kernel_spmd

    S1 = np.asarray(S1, np.float32)
    S2 = np.asarray(S2, np.float32)
    b, nk, _ = S1.shape
    _, nq, _ = S2.shape
    nc = _get_nc(nq, nk)

    # Fold weight pairs on host (f64 accumulation for accuracy)
    w1 = (np.asarray(Wq, np.float64) @ np.asarray(Wk, np.float64).T
          ).astype(np.float16)
    w2 = (np.asarray(Wv, np.float64) @ np.asarray(Wo, np.float64)
          ).astype(np.float16)
    w1_r = _tile_rows(w1, D // P)
    w2_r = _tile_rows(w2, D // P)
    bo_r = np.ascontiguousarray(
        np.asarray(bo, np.float32).reshape(D // P, P).T)  # [128, d_tiles]

    # key order = host->device staging order: critical tensors first
    in_maps = []
    for i in range(b):
        in_maps.append({
            "W1": w1_r,
            "S2T": _tile_rows(S2[i].T.astype(np.float16), D // P),
            "S1T": _tile_rows(S1[i].T.astype(np.float16), D // P),
            "W2": w2_r,
            "BO": bo_r,
            "S1B": _tile_rows(S1[i].astype(ml_dtypes.bfloat16), nk // P),
        })

    res = run_bass_kernel_spmd(nc, in_maps, list(range(b)), trace=_trace)
    out = np.stack([np.asarray(res.results[i]["OUT"]).T for i in range(b)])
    if _trace:
        kernel.last_result = res
    return np.ascontiguousarray(out.astype(np.float32))


# revision 9
# speedup vs baseline: 2.3836x; 1.0853x over previous
"""Cross-attention Trainium2 kernel (Bass/Tile), data-parallel over batch on 8 cores.

Reference computation per batch element b (no 1/sqrt(d) scaling):
    Q = S2[b] @ Wq; K = S1[b] @ Wk; V = S1[b] @ Wv
    A = softmax(Q @ K^T, -1)
    out[b] = (A @ V) @ Wo + bo

Low-rank reformulation (inner E = 1024 > D = 512, so fold the weight pairs):
    W1 = Wq @ Wk^T  [D, D]   (folded on host, f64 accumulation)
    W2 = Wv @ Wo    [D, D]
    scores = (S2 @ W1) @ S1^T     -- contraction D=512, not E=1024
    out    = (A @ S1) @ W2 + bo   -- contraction D=512, not E=1024
This halves the two big matmuls' contraction depth and removes the Q/K/V
projections entirely: ~380K PE cycles/core vs ~819K for the direct form.

Device layout is fully transposed (feature dims on SBUF partitions). All
inputs arrive pre-rearranged from the host in exactly the SBUF tile layout
([128 partitions, ...] dense per partition) so every DMA is a contiguous
per-partition copy. The scores chain (W1, S2T, TT, S1T) and output chain
(ZT, W2) run in fp16 (half the HBM bytes of f32, full PE rate, ~5e-4
element error); exp tiles stay bf16 for range (values up to e^61).

Per 512-query chunk:
    TT = W1-blocks^T @ S2T chunk          [d2, n]  (fp16)
    scoresT tiles [m-part, n-free] -> exp (no max subtraction: |score| <=
    ~70, exp stays in fp32 range) -> pairwise sum tree over the 16 exp
    tiles on the Pool engine -> single ones-matmul partition reduction ->
    reciprocal -> partition_broadcast -> ZT = S1-blocks^T @ E accumulated
    in 4 PSUM banks over all m tiles, normalized during eviction ->
    outT = W2-blocks^T @ ZT + bo -> DRAM [D, N2]; host transposes back.
"""
import sys

sys.path.insert(0, "/opt/trn_rl_repo")

import numpy as np
import ml_dtypes
from contextlib import ExitStack

P = 128
N_CORES = 8
B = 8          # batch (one element per core)
NQ = 2048      # queries (N2)
NK = 2048      # keys (N1)
D = 512        # query/cross dim
CHUNK = 512    # query-chunk width (moving free dim)

_cache = {}


def _build(nq=NQ, nk=NK):
    import concourse.tile as tile
    from concourse import bacc, mybir

    F32 = mybir.dt.float32
    F32R = mybir.dt.float32r
    F16 = mybir.dt.float16
    BF16 = mybir.dt.bfloat16
    Exp = mybir.ActivationFunctionType.Exp

    n_chunks = nq // CHUNK
    m_tiles = nk // P        # key tiles of 128 (16)
    d_tiles = D // P         # 4
    m_chunks = nk // CHUNK   # S1T load quarters (4)

    nc = bacc.Bacc("TRN2", target_bir_lowering=False, debug=False)

    # all dram tensors already in SBUF tile layout (dense per partition)
    S1T = nc.dram_tensor("S1T", [P, d_tiles, nk], F16, kind="ExternalInput").ap()
    S1B = nc.dram_tensor("S1B", [P, m_tiles, D], BF16, kind="ExternalInput").ap()
    S2T = nc.dram_tensor("S2T", [P, d_tiles, nq], F16, kind="ExternalInput").ap()
    W1 = nc.dram_tensor("W1", [P, d_tiles, D], F16, kind="ExternalInput").ap()
    W2 = nc.dram_tensor("W2", [P, d_tiles, D], F16, kind="ExternalInput").ap()
    BO = nc.dram_tensor("BO", [P, d_tiles], F32, kind="ExternalInput").ap()
    OUT = nc.dram_tensor("OUT", [D, nq], F32, kind="ExternalOutput").ap()

    with tile.TileContext(nc) as tc, ExitStack() as ctx, \
            nc.allow_low_precision(reason="fp16/bf16 staging for matmul operands"):
        const = ctx.enter_context(tc.tile_pool(name="const", bufs=1))
        w_pool = ctx.enter_context(tc.tile_pool(name="w_pool", bufs=1))
        tt_pool = ctx.enter_context(tc.tile_pool(name="tt_pool", bufs=2))
        e_pool = ctx.enter_context(tc.tile_pool(name="e_pool", bufs=m_tiles + 2))
        tree = ctx.enter_context(tc.tile_pool(name="tree", bufs=3))
        zt_pool = ctx.enter_context(tc.tile_pool(name="zt_pool", bufs=2))
        out_pool = ctx.enter_context(tc.tile_pool(name="out_pool", bufs=4))
        misc = ctx.enter_context(tc.tile_pool(name="misc", bufs=2))
        ps_mm = ctx.enter_context(tc.tile_pool(name="ps_mm", bufs=3, space="PSUM"))
        ps_z = ctx.enter_context(tc.tile_pool(name="ps_z", bufs=4, space="PSUM"))
        ps_sum = ctx.enter_context(tc.tile_pool(name="ps_sum", bufs=1, space="PSUM"))

        # constants
        ones_f = const.tile([P, 1], F32, name="ones_f")
        nc.any.memset(ones_f[:], 1.0)
        ones_r = const.tile([P, 1], F32R, name="ones_r")
        nc.vector.tensor_copy(ones_r[:], ones_f[:])
        ones_row_f = const.tile([1, P], F32, name="ones_row_f")
        nc.any.memset(ones_row_f[:], 1.0)
        ones_row = const.tile([1, P], F32R, name="ones_row")
        nc.vector.tensor_copy(ones_row[:], ones_row_f[:])
        wu_mov_f = const.tile([P, CHUNK], F32, name="wu_mov_f")
        nc.any.memset(wu_mov_f[:], 0.0)
        wu_mov = const.tile([P, CHUNK], F32R, name="wu_mov")
        nc.vector.tensor_copy(wu_mov[:], wu_mov_f[:])
        bo_t = const.tile([P, d_tiles], F32, name="bo_t")

        # HAM warmup: dummy matmuls fill the PE while the first input DMAs
        # are still in flight, so real matmuls start at the full 2.4 GHz
        for _ in range(16):
            wu_ps = ps_sum.tile([1, CHUNK], F32, name="wu_ps", tag="sum")
            nc.tensor.matmul(wu_ps[:], ones_r[:], wu_mov[:], start=True, stop=True)

        # persistent tensors
        w1_t = w_pool.tile([P, d_tiles, D], F16, name="w1_t")
        w2_t = w_pool.tile([P, d_tiles, D], F16, name="w2_t")
        s1t_res = w_pool.tile([P, d_tiles, nk], F16, name="s1t_res")
        s1b_res = w_pool.tile([P, m_tiles, D], BF16, name="s1b_res")
        s2_res = w_pool.tile([P, d_tiles, nq], F16, name="s2_res")

        # --- startup DMA: two HWDGE rings in parallel, ordered by the time
        # each tensor is first needed (w1/s2c0 -> TT0, s1t quarters ->
        # scores0, s1b halves -> ZT0, w2/bo -> out0, s2c1-3 -> later chunks)
        def _q(mc):
            return slice(mc * CHUNK, (mc + 1) * CHUNK)

        def _h(mh):
            return slice(mh * (m_tiles // 2), (mh + 1) * (m_tiles // 2))

        nc.sync.dma_start(w1_t[:], W1[:, :, :])
        nc.scalar.dma_start(s2_res[:, :, _q(0)], S2T[:, :, _q(0)])
        nc.sync.dma_start(s1t_res[:, :, _q(0)], S1T[:, :, _q(0)])
        nc.scalar.dma_start(s1t_res[:, :, _q(1)], S1T[:, :, _q(1)])
        nc.sync.dma_start(s1t_res[:, :, _q(2)], S1T[:, :, _q(2)])
        nc.scalar.dma_start(s1b_res[:, _h(0), :], S1B[:, _h(0), :])
        nc.sync.dma_start(s2_res[:, :, _q(1)], S2T[:, :, _q(1)])
        nc.scalar.dma_start(s1t_res[:, :, _q(3)], S1T[:, :, _q(3)])
        nc.sync.dma_start(s1b_res[:, _h(1), :], S1B[:, _h(1), :])
        nc.scalar.dma_start(w2_t[:], W2[:, :, :])
        nc.sync.dma_start(s2_res[:, :, _q(2)], S2T[:, :, _q(2)])
        nc.scalar.dma_start(bo_t[:], BO[:, :])
        nc.scalar.dma_start(s2_res[:, :, _q(3)], S2T[:, :, _q(3)])

        def emit_tt(c):
            """TT[d2, n] = sum_d1 W1[d1, d2] S2T[d1, n] for chunk c."""
            csl = slice(c * CHUNK, (c + 1) * CHUNK)
            tt_t = tt_pool.tile([P, d_tiles, CHUNK], F16, name="tt_t", tag="tt")
            for d2t in range(d_tiles):
                acc = ps_mm.tile([P, CHUNK], F32, name="accT", tag="mm")
                for d1t in range(d_tiles):
                    nc.tensor.matmul(
                        acc[:],
                        w1_t[:, d1t, d2t * P:(d2t + 1) * P],
                        s2_res[:, d1t, csl],
                        start=(d1t == 0), stop=(d1t == d_tiles - 1),
                    )
                nc.vector.tensor_copy(tt_t[:, d2t, :], acc[:])
            return tt_t

        def emit_out(c, zt_t):
            """outT[do, n] = sum_dz W2[dz, do] ZT[dz, n] + bo for chunk c."""
            csl = slice(c * CHUNK, (c + 1) * CHUNK)
            for dot in range(d_tiles):
                acc_o = ps_z.tile([P, CHUNK], F32, name="acc_o", tag="z")
                for dzt in range(d_tiles):
                    nc.tensor.matmul(
                        acc_o[:],
                        w2_t[:, dzt, dot * P:(dot + 1) * P],
                        zt_t[:, dzt, :],
                        start=(dzt == 0), stop=(dzt == d_tiles - 1),
                    )
                o_sb = out_pool.tile([P, CHUNK], F32, name="o_sb", tag="osb")
                nc.vector.tensor_scalar_add(o_sb[:], acc_o[:], bo_t[:, dot:dot + 1])
                eng = nc.sync if dot % 2 == 0 else nc.scalar
                eng.dma_start(OUT[dot * P:(dot + 1) * P, csl], o_sb[:])

        tt_t = emit_tt(0)
        for c in range(n_chunks):
          with nc.named_scope(f"chunk{c}"):
            # scoresT tiles + exp; pairwise sum tree on the Pool engine
            e_list = []
            lvl1 = [None] * 8
            lvl2 = [None] * 4
            lvl3 = [None] * 2
            s_all = None
            for mt in range(m_tiles):
                acc_s = ps_mm.tile([P, CHUNK], F32, name="acc_s", tag="mm")
                for d2t in range(d_tiles):
                    nc.tensor.matmul(
                        acc_s[:],
                        s1t_res[:, d2t, mt * P:(mt + 1) * P],
                        tt_t[:, d2t, :],
                        start=(d2t == 0), stop=(d2t == d_tiles - 1),
                    )
                e_t = e_pool.tile([P, CHUNK], BF16, name="e_t", tag="e")
                nc.scalar.activation(e_t[:], acc_s[:], Exp)
                e_list.append(e_t)
                if mt % 2 == 1:
                    # level-1 pair sums on the idle Pool engine; upper levels
                    # on DVE (Pool's per-op cost is ~2x DVE's)
                    k = mt // 2
                    t1 = tree.tile([P, CHUNK], F32R, name="t1", tag="t1")
                    nc.gpsimd.tensor_add(t1[:], e_list[mt - 1][:], e_list[mt][:])
                    lvl1[k] = t1
                    if k % 2 == 1:
                        j = k // 2
                        t2 = tree.tile([P, CHUNK], F32R, name="t2", tag="t2")
                        nc.vector.tensor_add(t2[:], lvl1[k - 1][:], lvl1[k][:])
                        lvl2[j] = t2
                        if j % 2 == 1:
                            i = j // 2
                            t3 = tree.tile([P, CHUNK], F32R, name="t3", tag="t3")
                            nc.vector.tensor_add(t3[:], lvl2[j - 1][:], lvl2[j][:])
                            lvl3[i] = t3
                            if i == 1:
                                s_all = tree.tile(
                                    [P, CHUNK], F32R, name="t4", tag="t4")
                                nc.vector.tensor_add(
                                    s_all[:], lvl3[0][:], lvl3[1][:])

            # ZT = S1^T @ E accumulated in 4 PSUM banks over all m tiles;
            # the single partition-reduction matmul for sumexp is slotted in
            # a few tiles into the loop (the Pool tree has finished by then)
            z_list = [
                ps_z.tile([P, CHUNK], F32, name="zt_ps", tag="z")
                for _ in range(d_tiles)
            ]
            sum_ps = None
            bc = None
            for mt in range(m_tiles):
                for dt_ in range(d_tiles):
                    nc.tensor.matmul(
                        z_list[dt_][:],
                        s1b_res[:, mt, dt_ * P:(dt_ + 1) * P],
                        e_list[mt][:],
                        start=(mt == 0), stop=(mt == m_tiles - 1),
                    )
                if mt == 8:
                    # partition reduction of the tree result (tree done ~mt 5)
                    sum_ps = ps_sum.tile([1, CHUNK], F32, name="sum_ps", tag="sum")
                    nc.tensor.matmul(
                        sum_ps[:], ones_r[:], s_all[:], start=True, stop=True)
                    sum_sb = misc.tile([1, CHUNK], F32, name="sum_sb", tag="sumsb")
                    nc.vector.tensor_copy(sum_sb[:], sum_ps[:])
                    recip = misc.tile([1, CHUNK], F32R, name="recip", tag="recip")
                    nc.vector.reciprocal(recip[:], sum_sb[:])
                if mt == 10:
                    # broadcast 1/sumexp to 128 partitions via a rank-1
                    # matmul (ones column x recip row) -- much faster than
                    # the gpsimd partition_broadcast ucode
                    bc_ps = ps_mm.tile([P, CHUNK], F32, name="bc_ps", tag="mm")
                    nc.tensor.matmul(
                        bc_ps[:], ones_row[:], recip[:], start=True, stop=True)
                    bc = misc.tile([P, CHUNK], F32, name="bc", tag="bc")
                    nc.vector.tensor_copy(bc[:], bc_ps[:])

            # DVE: normalize ZT out of PSUM first (out-proj is gated on it) ...
            zt_t = zt_pool.tile([P, d_tiles, CHUNK], F16, name="zt_t", tag="zt")
            for dt_ in range(d_tiles):
                nc.vector.tensor_mul(zt_t[:, dt_, :], z_list[dt_][:], bc[:])

            # ... while the PE runs the next chunk's TT, then our out-proj
            this_c = c
            if c + 1 < n_chunks:
                tt_t = emit_tt(c + 1)
            emit_out(this_c, zt_t)

    nc.compile()
    return nc


def _get_nc(nq=NQ, nk=NK):
    key = (nq, nk)
    if key not in _cache:
        _cache[key] = _build(nq, nk)
    return _cache[key]


def _tile_rows(a, t):
    """[t*128, X] row-major -> [128, t, X] (partition-major tile layout)."""
    x = a.shape[-1]
    return np.ascontiguousarray(a.reshape(t, P, x).transpose(1, 0, 2))


def kernel(S1, S2, Wq, Wk, Wv, Wo, bo, _trace=False):
    from concourse.bass_utils import run_bass_kernel_spmd

    S1 = np.asarray(S1, np.float32)
    S2 = np.asarray(S2, np.float32)
    b, nk, _ = S1.shape
    _, nq, _ = S2.shape
    nc = _get_nc(nq, nk)

    # Fold weight pairs on host (f64 accumulation for accuracy)
    w1 = (np.asarray(Wq, np.float64) @ np.asarray(Wk, np.float64).T
          ).astype(np.float16)
    w2 = (np.asarray(Wv, np.float64) @ np.asarray(Wo, np.float64)
          ).astype(np.float16)
    w1_r = _tile_rows(w1, D // P)
    w2_r = _tile_rows(w2, D // P)
    bo_r = np.ascontiguousarray(
        np.asarray(bo, np.float32).reshape(D // P, P).T)  # [128, d_tiles]

    # key order = host->device staging order: critical tensors first
    in_maps = []
    for i in range(b):
        in_maps.append({
            "W1": w1_r,
            "S2T": _tile_rows(S2[i].T.astype(np.float16), D // P),
            "S1T": _tile_rows(S1[i].T.astype(np.float16), D // P),
            "W2": w2_r,
            "BO": bo_r,
            "S1B": _tile_rows(S1[i].astype(ml_dtypes.bfloat16), nk // P),
        })

    res = run_bass_kernel_spmd(nc, in_maps, list(range(b)), trace=_trace)
    out = np.stack([np.asarray(res.results[i]["OUT"]).T for i in range(b)])
    if _trace:
        kernel.last_result = res
    return np.ascontiguousarray(out.astype(np.float32))
